# revision 1
# baseline (speedup 1.0000x reference)
"""MoE FFN (capacity-gated routing) on 8 Trainium2 NeuronCores.

Strategy
--------
Expert-parallel: 16 experts / 8 cores = 2 experts per core. With full
host-side I/O, the dispatch (which token goes to which expert slot) IS
the sharding step, so routing runs on the host in numpy; each core
receives only the tokens routed to its two experts, pre-gathered and
pre-transposed to d-major layout [E_per_core, D, CAP]. The device runs
the grouped FFN (x @ W1 -> exact GELU -> @ W2, biases fused into the
ScalarE activation as per-partition vectors) in bf16 with f32 PSUM
accumulation. The combine (scatter-add + 1/n_experts scaling) is a
cheap host-side gather. Tokens that overflow every attempted expert's
capacity (used == 0; zero of them for uniform-random routes) take the
fallback FFN, computed exactly on the host.
"""

import sys

if "/opt/trn_rl_repo" not in sys.path:
    sys.path.append("/opt/trn_rl_repo")

import numpy as np
import ml_dtypes

import concourse.tile as tile
from concourse import bacc, mybir
from concourse.bass_utils import run_bass_kernel_spmd

# Problem shape (hardcoded per contract)
D = 512        # d_model
F = 2048       # d_ff
E = 16         # num experts
B = 2048       # max tokens
CAP = 320      # per-expert capacity = int(1.25 * ceil(B * 2 / E))
N_CORES = 8
EPC = E // N_CORES  # experts per core

P = 128
KT = D // P    # k-tiles over d_model
FT = F // P    # tiles over d_ff

BF16 = mybir.dt.bfloat16
F32 = mybir.dt.float32
NPBF16 = ml_dtypes.bfloat16

_CACHE = {}


def _build_nc(rep=1):
    """Per-core program: grouped FFN for EPC experts over CAP token slots.

    All activations/weights are d-major (partition dim = feature dim), so
    b1/b2 are per-partition vectors and fuse into ScalarE activations.
    rep>1 repeats the whole body (for timing-by-differencing in test.py).
    """
    nc = bacc.Bacc(None)
    xg = nc.declare_dram_parameter("xg", [EPC, D, CAP], BF16, isOutput=False)
    w1 = nc.declare_dram_parameter("w1", [EPC, D, F], BF16, isOutput=False)
    b1 = nc.declare_dram_parameter("b1", [EPC, F], F32, isOutput=False)
    w2 = nc.declare_dram_parameter("w2", [EPC, F, D], BF16, isOutput=False)
    b2 = nc.declare_dram_parameter("b2", [EPC, D], F32, isOutput=False)
    yt = nc.declare_dram_parameter("yt", [EPC, D, CAP], F32, isOutput=True)

    with (
        tile.TileContext(nc) as tc,
        tc.tile_pool(name="consts", bufs=1) as consts,
        tc.tile_pool(name="xgp", bufs=2) as xgp,
        tc.tile_pool(name="w1p", bufs=2) as w1p,
        tc.tile_pool(name="w2p", bufs=2) as w2p,
        tc.tile_pool(name="hp", bufs=2) as hp,
        tc.tile_pool(name="yp", bufs=4) as yp,
        tc.tile_pool(name="ps1", bufs=4, space="PSUM") as ps1,
        tc.tile_pool(name="ps2", bufs=4, space="PSUM") as ps2,
    ):
        b1_sb = consts.tile([P, EPC, FT], F32)
        nc.sync.dma_start(out=b1_sb, in_=b1.rearrange("e (t p) -> p e t", p=P))
        b2_sb = consts.tile([P, EPC, KT], F32)
        nc.sync.dma_start(out=b2_sb, in_=b2.rearrange("e (t p) -> p e t", p=P))

        for _ in range(rep):
            for e in range(EPC):
                xg_sb = xgp.tile([P, KT, CAP], BF16)
                nc.sync.dma_start(
                    out=xg_sb, in_=xg[e].rearrange("(t p) n -> p t n", p=P)
                )
                w1_sb = w1p.tile([P, KT, F], BF16)
                nc.sync.dma_start(
                    out=w1_sb, in_=w1[e].rearrange("(t p) f -> p t f", p=P)
                )
                w2_sb = w2p.tile([P, FT, D], BF16)
                nc.sync.dma_start(
                    out=w2_sb, in_=w2[e].rearrange("(t p) d -> p t d", p=P)
                )
                h_sb = hp.tile([P, FT, CAP], BF16)
                for m in range(FT):
                    ps = ps1.tile([P, CAP], F32)
                    for k in range(KT):
                        nc.tensor.matmul(
                            ps,
                            w1_sb[:, k, m * P : (m + 1) * P],
                            xg_sb[:, k, :],
                            start=(k == 0),
                            stop=(k == KT - 1),
                        )
                    nc.scalar.activation(
                        h_sb[:, m, :],
                        ps,
                        mybir.ActivationFunctionType.Gelu,
                        bias=b1_sb[:, e, m : m + 1],
                    )
                for m2 in range(KT):
                    ps = ps2.tile([P, CAP], F32)
                    for t in range(FT):
                        nc.tensor.matmul(
                            ps,
                            w2_sb[:, t, m2 * P : (m2 + 1) * P],
                            h_sb[:, t, :],
                            start=(t == 0),
                            stop=(t == FT - 1),
                        )
                    y_sb = yp.tile([P, CAP], F32)
                    nc.scalar.activation(
                        y_sb,
                        ps,
                        mybir.ActivationFunctionType.Identity,
                        bias=b2_sb[:, e, m2 : m2 + 1],
                    )
                    nc.sync.dma_start(
                        out=yt[e, m2 * P : (m2 + 1) * P, :], in_=y_sb
                    )
    nc.finalize()
    return nc


def get_nc(rep=1):
    if rep not in _CACHE:
        _CACHE[rep] = _build_nc(rep)
    return _CACHE[rep]


def _route_np(routes):
    """Numpy replica of the reference's capacity-gated routing."""
    e_map = (routes.astype(np.int64) * E) // B                  # [B, K]
    sel0 = np.zeros((B, E), bool)
    np.put_along_axis(sel0, e_map, True, axis=1)
    sel0_i = sel0.astype(np.int32)
    cum = np.cumsum(sel0_i, axis=0) - sel0_i                    # exclusive cumsum
    selected = sel0 & (cum < CAP)
    slot = cum
    used = selected.sum(axis=1)
    tok_of_slot = np.zeros(E * CAP, np.int32)
    valid = np.zeros(E * CAP, bool)
    b_idx, e_idx = np.nonzero(selected)
    flat = e_idx * CAP + slot[b_idx, e_idx]
    tok_of_slot[flat] = b_idx
    valid[flat] = True
    return tok_of_slot, valid, used, selected, slot


def _erf(v):
    # Abramowitz & Stegun 7.1.26, |err| <= 1.5e-7
    s = np.sign(v)
    a = np.abs(v)
    t = 1.0 / (1.0 + 0.3275911 * a)
    poly = t * (
        0.254829592
        + t * (-0.284496736 + t * (1.421413741 + t * (-1.453152027 + t * 1.061405429)))
    )
    return s * (1.0 - poly * np.exp(-a * a))


def _gelu_exact(v):
    return 0.5 * v * (1.0 + _erf(v / np.sqrt(2.0)))


def kernel(x, W1, b1, W2, b2, Wf1, bf1, Wf2, bf2, routes):
    x = np.asarray(x, np.float32)
    W1 = np.asarray(W1, np.float32)
    b1 = np.asarray(b1, np.float32)
    W2 = np.asarray(W2, np.float32)
    b2 = np.asarray(b2, np.float32)
    Wf1 = np.asarray(Wf1, np.float32)
    bf1 = np.asarray(bf1, np.float32)
    Wf2 = np.asarray(Wf2, np.float32)
    bf2 = np.asarray(bf2, np.float32)
    routes = np.asarray(routes)

    tok_of_slot, valid, used, selected, slot = _route_np(routes)

    # Dispatch (= the sharding step): gather tokens into per-expert
    # capacity buffers, transpose to d-major, cast to bf16.
    xg = x[tok_of_slot] * valid[:, None].astype(np.float32)     # [E*CAP, D]
    xgT = np.ascontiguousarray(
        xg.reshape(E, CAP, D).transpose(0, 2, 1)                # [E, D, CAP]
    ).astype(NPBF16)
    w1T = W1.astype(NPBF16)                                     # [E, D, F] (natural lhsT layout)
    w2T = W2.astype(NPBF16)                                     # [E, F, D]

    in_maps = []
    for c in range(N_CORES):
        sl = slice(c * EPC, (c + 1) * EPC)
        in_maps.append(
            {
                "xg": np.ascontiguousarray(xgT[sl]),
                "w1": np.ascontiguousarray(w1T[sl]),
                "b1": np.ascontiguousarray(b1[sl]),
                "w2": np.ascontiguousarray(w2T[sl]),
                "b2": np.ascontiguousarray(b2[sl]),
            }
        )

    nc = get_nc()
    res = run_bass_kernel_spmd(nc, in_maps, core_ids=list(range(N_CORES)))

    # [E, D, CAP] expert outputs (garbage in invalid slots; never read there)
    Y = np.concatenate([r["yt"] for r in res.results], axis=0)

    # Combine: each token was selected by <= 2 experts; gather its slot
    # outputs and average. Pure host-side gather.
    b_idx, e_idx = np.nonzero(selected)                         # ordered by token
    first = np.concatenate(([True], b_idx[1:] != b_idx[:-1]))
    s_of = slot[b_idx, e_idx]
    e1 = np.zeros(B, np.int64); c1 = np.zeros(B, np.int64); g1 = np.zeros(B, np.float32)
    e2 = np.zeros(B, np.int64); c2 = np.zeros(B, np.int64); g2 = np.zeros(B, np.float32)
    e1[b_idx[first]] = e_idx[first]; c1[b_idx[first]] = s_of[first]; g1[b_idx[first]] = 1.0
    e2[b_idx[~first]] = e_idx[~first]; c2[b_idx[~first]] = s_of[~first]; g2[b_idx[~first]] = 1.0
    out_sum = g1[:, None] * Y[e1, :, c1] + g2[:, None] * Y[e2, :, c2]
    inv = (1.0 / np.maximum(used, 1)).astype(np.float32)
    out = out_sum * inv[:, None]

    # Overflow tokens (used == 0): exact fallback FFN on host.
    ovf = np.nonzero(used == 0)[0]
    if ovf.size:
        xo = x[ovf]
        fb = _gelu_exact(xo @ Wf1 + bf1) @ Wf2 + bf2
        out[ovf] = fb.astype(np.float32)

    return out.astype(np.float32)


# revision 12
# speedup vs baseline: 5065.4404x; 5065.4404x over previous
"""MoE FFN (capacity-gated routing) on 8 Trainium2 NeuronCores.

Strategy
--------
Expert-parallel: 16 experts / 8 cores = 2 experts per core. With full
host-side I/O, the dispatch (which token goes to which expert slot) IS
the sharding step, so routing runs on the host in numpy; each core
receives only the tokens routed to its two experts, pre-gathered,
pre-transposed to d-major layout, and pre-packed into the exact
per-partition SBUF byte order so every weight DMA is a contiguous
[128 x 4KB] transfer at full HBM bandwidth. The device runs the grouped
FFN (x @ W1 -> exact GELU -> @ W2, biases fused into ScalarE
activations as per-partition vectors) in bf16 with f32 PSUM
accumulation. Weight DMAs are chunked (4 chunks per matrix) so the
TensorE starts ~2us into the kernel and weights stream concurrently
with compute. The combine (scatter-add + 1/n_experts scaling) is a
cheap host-side gather. Tokens that overflow every attempted expert's
capacity (used == 0; zero of them for uniform-random routes) take the
fallback FFN, computed exactly on the host.
"""

import sys

if "/opt/trn_rl_repo" not in sys.path:
    sys.path.append("/opt/trn_rl_repo")

import numpy as np
import ml_dtypes

import concourse.tile as tile
from concourse import bacc, mybir
from concourse.bass_utils import run_bass_kernel_spmd

# Problem shape (hardcoded per contract)
D = 512        # d_model
F = 2048       # d_ff
E = 16         # num experts
B = 2048       # max tokens
CAP = 320      # per-expert capacity = int(1.25 * ceil(B * 2 / E))
N_CORES = 8
EPC = E // N_CORES  # experts per core

P = 128
KT = D // P    # k-tiles over d_model
FT = F // P    # tiles over d_ff
CH1 = 8        # w1 chunks along f
CH2 = 4        # w2 chunks along t (contraction)
M_PER = FT // CH1
T_PER = FT // CH2

BF16 = mybir.dt.bfloat16
F32 = mybir.dt.float32
NPBF16 = ml_dtypes.bfloat16

_CACHE = {}


def _build_nc(rep=1, act=None):
    """Per-core program: grouped FFN for EPC experts over CAP token slots.

    All tensors are d-major (partition dim = feature dim); weights arrive
    pre-packed in per-partition byte order (see _prep_in_maps), so every
    DMA is [128 x contiguous]. rep>1 repeats the body (timing); act
    overrides the GELU (CoreSim lacks Gelu; Identity is cost-equivalent).
    """
    if act is None:
        act = mybir.ActivationFunctionType.Gelu
    nc = bacc.Bacc(None)
    xg = nc.declare_dram_parameter("xg", [EPC, P, KT * CAP], BF16, isOutput=False)
    w1 = nc.declare_dram_parameter(
        "w1", [EPC, CH1, P, KT * M_PER * P], BF16, isOutput=False
    )
    w2 = nc.declare_dram_parameter(
        "w2", [EPC, CH2, P, T_PER * D], BF16, isOutput=False
    )
    bb = nc.declare_dram_parameter("bb", [EPC, F + D], F32, isOutput=False)
    yt = nc.declare_dram_parameter("yt", [EPC, P, KT * CAP], F32, isOutput=True)

    with (
        tile.TileContext(nc) as tc,
        tc.tile_pool(name="consts", bufs=1) as consts,
        tc.tile_pool(name="xgp", bufs=2) as xgp,
        tc.tile_pool(name="w1p", bufs=2 * CH1 + 1) as w1p,
        tc.tile_pool(name="w2p", bufs=2 * CH2 + 1) as w2p,
        tc.tile_pool(name="hp", bufs=2) as hp,
        tc.tile_pool(name="yp", bufs=2) as yp,
        tc.tile_pool(name="ps1", bufs=4, space="PSUM") as ps1,
        tc.tile_pool(name="ps2", bufs=4, space="PSUM") as ps2,
    ):
        bb_sb = consts.tile([P, EPC, FT + KT], F32)
        bias_issued = False

        for _ in range(rep):
            for e in range(EPC):
                w1cs = []
                for c in range(CH1):
                    w1c = w1p.tile([P, KT, M_PER * P], BF16, tag="w1c", name="w1c")
                    if c == 0:
                        # first weight chunk ahead of everything it gates
                        nc.sync.dma_start(
                            out=w1c,
                            in_=w1[e, 0].rearrange("p (t f) -> p t f", t=KT),
                        )
                    w1cs.append(w1c)
                xg_sb = xgp.tile([P, KT, CAP], BF16, name="xg_sb")
                nc.sync.dma_start(
                    out=xg_sb, in_=xg[e].rearrange("p (t n) -> p t n", t=KT)
                )
                h_sb = hp.tile([P, FT, CAP], BF16, name="h_sb")

                # phase 1: h = gelu(x @ W1 + b1); w1 chunked along f so the
                # first matmuls start after one 512 KB chunk lands
                for c in range(CH1):
                    w1c = w1cs[c]
                    if c > 0:
                        nc.sync.dma_start(
                            out=w1c,
                            in_=w1[e, c].rearrange("p (t f) -> p t f", t=KT),
                        )
                    for ml in range(M_PER):
                        m = c * M_PER + ml
                        ps = ps1.tile([P, CAP], F32, name="ps")
                        for k in range(KT):
                            nc.tensor.matmul(
                                ps,
                                w1c[:, k, ml * P : (ml + 1) * P],
                                xg_sb[:, k, :],
                                start=(k == 0),
                                stop=(k == KT - 1),
                            )
                        if not bias_issued:
                            # traced before the first ACT (deps follow trace
                            # order) but after the critical xg/w1c0 loads
                            nc.sync.dma_start(
                                out=bb_sb, in_=bb.rearrange("e (t p) -> p e t", p=P)
                            )
                            bias_issued = True
                        nc.scalar.activation(
                            h_sb[:, m, :],
                            ps,
                            act,
                            bias=bb_sb[:, e, m : m + 1],
                        )

                # phase 2: y = h @ W2 + b2; t-outer so each w2 chunk is
                # consumed on arrival; the 4 output groups accumulate in
                # parallel across the 4 ps2 banks
                ps_y = [
                    ps2.tile([P, CAP], F32, tag="psy", name=f"psy{m2}")
                    for m2 in range(KT)
                ]
                w2cs = []
                for c in range(CH2):
                    w2c = w2p.tile([P, T_PER, D], BF16, tag="w2c", name="w2c")
                    nc.sync.dma_start(
                        out=w2c,
                        in_=w2[e, c].rearrange("p (t d) -> p t d", t=T_PER),
                    )
                    w2cs.append(w2c)

                def mm2(t, m2):
                    c, tl = divmod(t, T_PER)
                    nc.tensor.matmul(
                        ps_y[m2],
                        w2cs[c][:, tl, m2 * P : (m2 + 1) * P],
                        h_sb[:, t, :],
                        start=(t == 0),
                        stop=(t == FT - 1),
                    )

                y_sb = yp.tile([P, KT, CAP], F32, name="y_sb")

                def act_y(m2):
                    nc.scalar.activation(
                        y_sb[:, m2, :],
                        ps_y[m2],
                        mybir.ActivationFunctionType.Identity,
                        bias=bb_sb[:, e, FT + m2 : FT + m2 + 1],
                    )

                last = e == EPC - 1
                half = FT // 2
                for t in range(half if last else FT):
                    for m2 in range(KT):
                        mm2(t, m2)
                if not last:
                    for m2 in range(KT):
                        act_y(m2)
                    nc.sync.dma_start(
                        out=yt[e].rearrange("p (t n) -> p t n", t=KT), in_=y_sb
                    )
                else:
                    # m2-outer second half: groups retire one by one so the
                    # two half-DMAs overlap the tail matmuls
                    for m2 in range(KT):
                        for t in range(half, FT):
                            mm2(t, m2)
                        act_y(m2)
                        if m2 == 1:
                            nc.sync.dma_start(
                                out=yt[e].rearrange("p (t n) -> p t n", t=KT)[
                                    :, 0:2, :
                                ],
                                in_=y_sb[:, 0:2, :],
                            )
                    nc.sync.dma_start(
                        out=yt[e].rearrange("p (t n) -> p t n", t=KT)[:, 2:4, :],
                        in_=y_sb[:, 2:4, :],
                    )
    nc.finalize()
    return nc


def get_nc(rep=1, act=None):
    key = (rep, act)
    if key not in _CACHE:
        _CACHE[key] = _build_nc(rep, act)
    return _CACHE[key]


def _route_np(routes):
    """Numpy replica of the reference's capacity-gated routing."""
    e_map = (routes.astype(np.int64) * E) // B                  # [B, K]
    sel0 = np.zeros((B, E), bool)
    np.put_along_axis(sel0, e_map, True, axis=1)
    sel0_i = sel0.astype(np.int32)
    cum = np.cumsum(sel0_i, axis=0) - sel0_i                    # exclusive cumsum
    selected = sel0 & (cum < CAP)
    slot = cum
    used = selected.sum(axis=1)
    tok_of_slot = np.zeros(E * CAP, np.int32)
    valid = np.zeros(E * CAP, bool)
    b_idx, e_idx = np.nonzero(selected)
    flat = e_idx * CAP + slot[b_idx, e_idx]
    tok_of_slot[flat] = b_idx
    valid[flat] = True
    return tok_of_slot, valid, used, selected, slot


def _pack_w1(W1e):
    # [D, F] -> [CH1, P, KT*M_PER*P]; [c, p, k*512 + x] = W1e[k*128+p, c*512+x]
    return np.ascontiguousarray(
        W1e.reshape(KT, P, CH1, M_PER * P).transpose(2, 1, 0, 3).reshape(CH1, P, -1)
    )


def _pack_w2(W2e):
    # [F, D] -> [CH2, P, T_PER*D]; [c, p, tl*512 + d] = W2e[(c*T_PER+tl)*128+p, d]
    return np.ascontiguousarray(
        W2e.reshape(CH2, T_PER, P, D).transpose(0, 2, 1, 3).reshape(CH2, P, -1)
    )


def _prep_in_maps(x, W1, b1, W2, b2, routing):
    tok_of_slot, valid, used, selected, slot = routing
    xg = x[tok_of_slot] * valid[:, None].astype(np.float32)     # [E*CAP, D]
    xgT = xg.reshape(E, CAP, D).transpose(0, 2, 1)              # [E, D, CAP]
    xg_pk = np.ascontiguousarray(
        xgT.reshape(E, KT, P, CAP).transpose(0, 2, 1, 3).reshape(E, P, -1)
    ).astype(NPBF16)
    w1_pk = W1.astype(NPBF16)
    w2_pk = W2.astype(NPBF16)
    in_maps = []
    for c in range(N_CORES):
        sl = slice(c * EPC, (c + 1) * EPC)
        in_maps.append(
            {
                "xg": xg_pk[sl],
                "w1": np.stack([_pack_w1(w1_pk[i]) for i in range(sl.start, sl.stop)]),
                "w2": np.stack([_pack_w2(w2_pk[i]) for i in range(sl.start, sl.stop)]),
                "bb": np.ascontiguousarray(np.concatenate([b1[sl], b2[sl]], axis=1)),
            }
        )
    return in_maps


def _erf(v):
    # Abramowitz & Stegun 7.1.26, |err| <= 1.5e-7
    s = np.sign(v)
    a = np.abs(v)
    t = 1.0 / (1.0 + 0.3275911 * a)
    poly = t * (
        0.254829592
        + t * (-0.284496736 + t * (1.421413741 + t * (-1.453152027 + t * 1.061405429)))
    )
    return s * (1.0 - poly * np.exp(-a * a))


def _gelu_exact(v):
    return 0.5 * v * (1.0 + _erf(v / np.sqrt(2.0)))


def kernel(x, W1, b1, W2, b2, Wf1, bf1, Wf2, bf2, routes):
    x = np.asarray(x, np.float32)
    W1 = np.asarray(W1, np.float32)
    b1 = np.asarray(b1, np.float32)
    W2 = np.asarray(W2, np.float32)
    b2 = np.asarray(b2, np.float32)
    Wf1 = np.asarray(Wf1, np.float32)
    bf1 = np.asarray(bf1, np.float32)
    Wf2 = np.asarray(Wf2, np.float32)
    bf2 = np.asarray(bf2, np.float32)
    routes = np.asarray(routes)

    routing = _route_np(routes)
    tok_of_slot, valid, used, selected, slot = routing
    in_maps = _prep_in_maps(x, W1, b1, W2, b2, routing)

    nc = get_nc()
    res = run_bass_kernel_spmd(nc, in_maps, core_ids=list(range(N_CORES)))

    # [E, D, CAP] expert outputs (garbage in invalid slots; never read there)
    Y = np.concatenate([r["yt"] for r in res.results], axis=0)
    Y = Y.reshape(E, P, KT, CAP).transpose(0, 2, 1, 3).reshape(E, D, CAP)

    # Combine: each token was selected by <= 2 experts; gather its slot
    # outputs and average. Pure host-side gather.
    b_idx, e_idx = np.nonzero(selected)                         # ordered by token
    first = np.concatenate(([True], b_idx[1:] != b_idx[:-1]))
    s_of = slot[b_idx, e_idx]
    e1 = np.zeros(B, np.int64); c1 = np.zeros(B, np.int64); g1 = np.zeros(B, np.float32)
    e2 = np.zeros(B, np.int64); c2 = np.zeros(B, np.int64); g2 = np.zeros(B, np.float32)
    e1[b_idx[first]] = e_idx[first]; c1[b_idx[first]] = s_of[first]; g1[b_idx[first]] = 1.0
    e2[b_idx[~first]] = e_idx[~first]; c2[b_idx[~first]] = s_of[~first]; g2[b_idx[~first]] = 1.0
    out_sum = g1[:, None] * Y[e1, :, c1] + g2[:, None] * Y[e2, :, c2]
    inv = (1.0 / np.maximum(used, 1)).astype(np.float32)
    out = out_sum * inv[:, None]

    # Overflow tokens (used == 0): exact fallback FFN on host.
    ovf = np.nonzero(used == 0)[0]
    if ovf.size:
        xo = x[ovf]
        fb = _gelu_exact(xo @ Wf1 + bf1) @ Wf2 + bf2
        out[ovf] = fb.astype(np.float32)

    return out.astype(np.float32)


# revision 13
# speedup vs baseline: 5508.7141x; 1.0875x over previous
"""MoE FFN (capacity-gated routing) on 8 Trainium2 NeuronCores.

Strategy
--------
Expert-parallel: 16 experts / 8 cores = 2 experts per core. With full
host-side I/O, the dispatch (which token goes to which expert slot) IS
the sharding step, so routing runs on the host in numpy; each core
receives only the tokens routed to its two experts, pre-gathered,
pre-transposed to d-major layout, and pre-packed into the exact
per-partition SBUF byte order so every weight DMA is a contiguous
[128 x 4KB] transfer at full HBM bandwidth. The device runs the grouped
FFN (x @ W1 -> exact GELU -> @ W2, biases fused into ScalarE
activations as per-partition vectors) in bf16 with f32 PSUM
accumulation. Weight DMAs are chunked (4 chunks per matrix) so the
TensorE starts ~2us into the kernel and weights stream concurrently
with compute. The combine (scatter-add + 1/n_experts scaling) is a
cheap host-side gather. Tokens that overflow every attempted expert's
capacity (used == 0; zero of them for uniform-random routes) take the
fallback FFN, computed exactly on the host.
"""

import sys

if "/opt/trn_rl_repo" not in sys.path:
    sys.path.append("/opt/trn_rl_repo")

import numpy as np
import ml_dtypes

import concourse.tile as tile
from concourse import bacc, mybir
from concourse.bass_utils import run_bass_kernel_spmd

# Problem shape (hardcoded per contract)
D = 512        # d_model
F = 2048       # d_ff
E = 16         # num experts
B = 2048       # max tokens
CAP = 320      # per-expert capacity = int(1.25 * ceil(B * 2 / E))
N_CORES = 8
EPC = E // N_CORES  # experts per core

P = 128
KT = D // P    # k-tiles over d_model
FT = F // P    # tiles over d_ff
CH1 = 8        # w1 chunks along f
CH2 = 4        # w2 chunks along t (contraction)
M_PER = FT // CH1
T_PER = FT // CH2

BF16 = mybir.dt.bfloat16
F32 = mybir.dt.float32
NPBF16 = ml_dtypes.bfloat16

_CACHE = {}


def _build_nc(rep=1, act=None, nb=CAP):
    """Per-core program: grouped FFN for EPC experts over CAP token slots.

    All tensors are d-major (partition dim = feature dim); weights arrive
    pre-packed in per-partition byte order (see _prep_in_maps), so every
    DMA is [128 x contiguous]. rep>1 repeats the body (timing); act
    overrides the GELU (CoreSim lacks Gelu; Identity is cost-equivalent).
    """
    if act is None:
        act = mybir.ActivationFunctionType.Gelu
    nc = bacc.Bacc(None)
    xg = nc.declare_dram_parameter("xg", [EPC, P, KT * nb], BF16, isOutput=False)
    w1 = nc.declare_dram_parameter(
        "w1", [EPC, CH1, P, KT * M_PER * P], BF16, isOutput=False
    )
    w2 = nc.declare_dram_parameter(
        "w2", [EPC, CH2, P, T_PER * D], BF16, isOutput=False
    )
    bb = nc.declare_dram_parameter("bb", [EPC, F + D], F32, isOutput=False)
    yt = nc.declare_dram_parameter("yt", [EPC, P, KT * nb], F32, isOutput=True)

    with (
        tile.TileContext(nc) as tc,
        tc.tile_pool(name="consts", bufs=1) as consts,
        tc.tile_pool(name="xgp", bufs=2) as xgp,
        tc.tile_pool(name="w1p", bufs=2 * CH1 + 1) as w1p,
        tc.tile_pool(name="w2p", bufs=2 * CH2 + 1) as w2p,
        tc.tile_pool(name="hp", bufs=2) as hp,
        tc.tile_pool(name="yp", bufs=2) as yp,
        tc.tile_pool(name="ps1", bufs=4, space="PSUM") as ps1,
        tc.tile_pool(name="ps2", bufs=4, space="PSUM") as ps2,
    ):
        bb_sb = consts.tile([P, EPC, FT + KT], F32)
        bias_issued = False

        for _ in range(rep):
            for e in range(EPC):
                w1cs = []
                for c in range(CH1):
                    w1c = w1p.tile([P, KT, M_PER * P], BF16, tag="w1c", name="w1c")
                    if c == 0:
                        # first weight chunk ahead of everything it gates
                        nc.sync.dma_start(
                            out=w1c,
                            in_=w1[e, 0].rearrange("p (t f) -> p t f", t=KT),
                        )
                    w1cs.append(w1c)
                xg_sb = xgp.tile([P, KT, nb], BF16, name="xg_sb")
                nc.sync.dma_start(
                    out=xg_sb, in_=xg[e].rearrange("p (t n) -> p t n", t=KT)
                )
                h_sb = hp.tile([P, FT, nb], BF16, name="h_sb")

                # phase 1: h = gelu(x @ W1 + b1); w1 chunked along f so the
                # first matmuls start after one 512 KB chunk lands
                for c in range(CH1):
                    w1c = w1cs[c]
                    if c > 0:
                        nc.sync.dma_start(
                            out=w1c,
                            in_=w1[e, c].rearrange("p (t f) -> p t f", t=KT),
                        )
                    for ml in range(M_PER):
                        m = c * M_PER + ml
                        ps = ps1.tile([P, nb], F32, name="ps")
                        for k in range(KT):
                            nc.tensor.matmul(
                                ps,
                                w1c[:, k, ml * P : (ml + 1) * P],
                                xg_sb[:, k, :],
                                start=(k == 0),
                                stop=(k == KT - 1),
                            )
                        if not bias_issued:
                            # traced before the first ACT (deps follow trace
                            # order) but after the critical xg/w1c0 loads
                            nc.sync.dma_start(
                                out=bb_sb, in_=bb.rearrange("e (t p) -> p e t", p=P)
                            )
                            bias_issued = True
                        nc.scalar.activation(
                            h_sb[:, m, :],
                            ps,
                            act,
                            bias=bb_sb[:, e, m : m + 1],
                        )

                # phase 2: y = h @ W2 + b2; t-outer so each w2 chunk is
                # consumed on arrival; the 4 output groups accumulate in
                # parallel across the 4 ps2 banks
                ps_y = [
                    ps2.tile([P, nb], F32, tag="psy", name=f"psy{m2}")
                    for m2 in range(KT)
                ]
                w2cs = []
                for c in range(CH2):
                    w2c = w2p.tile([P, T_PER, D], BF16, tag="w2c", name="w2c")
                    nc.sync.dma_start(
                        out=w2c,
                        in_=w2[e, c].rearrange("p (t d) -> p t d", t=T_PER),
                    )
                    w2cs.append(w2c)

                def mm2(t, m2):
                    c, tl = divmod(t, T_PER)
                    nc.tensor.matmul(
                        ps_y[m2],
                        w2cs[c][:, tl, m2 * P : (m2 + 1) * P],
                        h_sb[:, t, :],
                        start=(t == 0),
                        stop=(t == FT - 1),
                    )

                y_sb = yp.tile([P, KT, nb], F32, name="y_sb")

                def act_y(m2):
                    nc.scalar.activation(
                        y_sb[:, m2, :],
                        ps_y[m2],
                        mybir.ActivationFunctionType.Identity,
                        bias=bb_sb[:, e, FT + m2 : FT + m2 + 1],
                    )

                last = e == EPC - 1
                half = FT // 2
                for t in range(half if last else FT):
                    for m2 in range(KT):
                        mm2(t, m2)
                if not last:
                    for m2 in range(KT):
                        act_y(m2)
                    nc.sync.dma_start(
                        out=yt[e].rearrange("p (t n) -> p t n", t=KT), in_=y_sb
                    )
                else:
                    # m2-outer second half: groups retire one by one so the
                    # two half-DMAs overlap the tail matmuls
                    for m2 in range(KT):
                        for t in range(half, FT):
                            mm2(t, m2)
                        act_y(m2)
                        if m2 == 1:
                            nc.sync.dma_start(
                                out=yt[e].rearrange("p (t n) -> p t n", t=KT)[
                                    :, 0:2, :
                                ],
                                in_=y_sb[:, 0:2, :],
                            )
                    nc.sync.dma_start(
                        out=yt[e].rearrange("p (t n) -> p t n", t=KT)[:, 2:4, :],
                        in_=y_sb[:, 2:4, :],
                    )
    nc.finalize()
    return nc


def get_nc(rep=1, act=None, nb=CAP):
    key = (rep, act, nb)
    if key not in _CACHE:
        _CACHE[key] = _build_nc(rep, act, nb)
    return _CACHE[key]


def _bucket(routing):
    """Token-block width: smallest multiple of 32 covering the max expert
    load (valid slots are dense per expert), floored at 192 to bound the
    number of distinct compiled programs."""
    selected = routing[3]
    maxload = int(selected.sum(axis=0).max())
    nb = max(192, -(-maxload // 32) * 32)
    return min(CAP, nb)


def _route_np(routes):
    """Numpy replica of the reference's capacity-gated routing."""
    e_map = (routes.astype(np.int64) * E) // B                  # [B, K]
    sel0 = np.zeros((B, E), bool)
    np.put_along_axis(sel0, e_map, True, axis=1)
    sel0_i = sel0.astype(np.int32)
    cum = np.cumsum(sel0_i, axis=0) - sel0_i                    # exclusive cumsum
    selected = sel0 & (cum < CAP)
    slot = cum
    used = selected.sum(axis=1)
    tok_of_slot = np.zeros(E * CAP, np.int32)
    valid = np.zeros(E * CAP, bool)
    b_idx, e_idx = np.nonzero(selected)
    flat = e_idx * CAP + slot[b_idx, e_idx]
    tok_of_slot[flat] = b_idx
    valid[flat] = True
    return tok_of_slot, valid, used, selected, slot


def _pack_w1(W1e):
    # [D, F] -> [CH1, P, KT*M_PER*P]; [c, p, k*512 + x] = W1e[k*128+p, c*512+x]
    return np.ascontiguousarray(
        W1e.reshape(KT, P, CH1, M_PER * P).transpose(2, 1, 0, 3).reshape(CH1, P, -1)
    )


def _pack_w2(W2e):
    # [F, D] -> [CH2, P, T_PER*D]; [c, p, tl*512 + d] = W2e[(c*T_PER+tl)*128+p, d]
    return np.ascontiguousarray(
        W2e.reshape(CH2, T_PER, P, D).transpose(0, 2, 1, 3).reshape(CH2, P, -1)
    )


def _prep_in_maps(x, W1, b1, W2, b2, routing, nb=CAP):
    tok_of_slot, valid, used, selected, slot = routing
    xg = x[tok_of_slot] * valid[:, None].astype(np.float32)     # [E*CAP, D]
    xgT = xg.reshape(E, CAP, D).transpose(0, 2, 1)[:, :, :nb]   # [E, D, nb]
    xg_pk = np.ascontiguousarray(
        xgT.reshape(E, KT, P, nb).transpose(0, 2, 1, 3).reshape(E, P, -1)
    ).astype(NPBF16)
    w1_pk = W1.astype(NPBF16)
    w2_pk = W2.astype(NPBF16)
    in_maps = []
    for c in range(N_CORES):
        sl = slice(c * EPC, (c + 1) * EPC)
        in_maps.append(
            {
                "xg": xg_pk[sl],
                "w1": np.stack([_pack_w1(w1_pk[i]) for i in range(sl.start, sl.stop)]),
                "w2": np.stack([_pack_w2(w2_pk[i]) for i in range(sl.start, sl.stop)]),
                "bb": np.ascontiguousarray(np.concatenate([b1[sl], b2[sl]], axis=1)),
            }
        )
    return in_maps


def _erf(v):
    # Abramowitz & Stegun 7.1.26, |err| <= 1.5e-7
    s = np.sign(v)
    a = np.abs(v)
    t = 1.0 / (1.0 + 0.3275911 * a)
    poly = t * (
        0.254829592
        + t * (-0.284496736 + t * (1.421413741 + t * (-1.453152027 + t * 1.061405429)))
    )
    return s * (1.0 - poly * np.exp(-a * a))


def _gelu_exact(v):
    return 0.5 * v * (1.0 + _erf(v / np.sqrt(2.0)))


def kernel(x, W1, b1, W2, b2, Wf1, bf1, Wf2, bf2, routes):
    x = np.asarray(x, np.float32)
    W1 = np.asarray(W1, np.float32)
    b1 = np.asarray(b1, np.float32)
    W2 = np.asarray(W2, np.float32)
    b2 = np.asarray(b2, np.float32)
    Wf1 = np.asarray(Wf1, np.float32)
    bf1 = np.asarray(bf1, np.float32)
    Wf2 = np.asarray(Wf2, np.float32)
    bf2 = np.asarray(bf2, np.float32)
    routes = np.asarray(routes)

    routing = _route_np(routes)
    tok_of_slot, valid, used, selected, slot = routing
    nb = _bucket(routing)
    in_maps = _prep_in_maps(x, W1, b1, W2, b2, routing, nb)

    nc = get_nc(nb=nb)
    res = run_bass_kernel_spmd(nc, in_maps, core_ids=list(range(N_CORES)))

    # [E, D, CAP] expert outputs (garbage in invalid slots; never read there)
    Y = np.concatenate([r["yt"] for r in res.results], axis=0)
    Y = Y.reshape(E, P, KT, nb).transpose(0, 2, 1, 3).reshape(E, D, nb)

    # Combine: each token was selected by <= 2 experts; gather its slot
    # outputs and average. Pure host-side gather.
    b_idx, e_idx = np.nonzero(selected)                         # ordered by token
    first = np.concatenate(([True], b_idx[1:] != b_idx[:-1]))
    s_of = slot[b_idx, e_idx]
    e1 = np.zeros(B, np.int64); c1 = np.zeros(B, np.int64); g1 = np.zeros(B, np.float32)
    e2 = np.zeros(B, np.int64); c2 = np.zeros(B, np.int64); g2 = np.zeros(B, np.float32)
    e1[b_idx[first]] = e_idx[first]; c1[b_idx[first]] = s_of[first]; g1[b_idx[first]] = 1.0
    e2[b_idx[~first]] = e_idx[~first]; c2[b_idx[~first]] = s_of[~first]; g2[b_idx[~first]] = 1.0
    out_sum = g1[:, None] * Y[e1, :, c1] + g2[:, None] * Y[e2, :, c2]
    inv = (1.0 / np.maximum(used, 1)).astype(np.float32)
    out = out_sum * inv[:, None]

    # Overflow tokens (used == 0): exact fallback FFN on host.
    ovf = np.nonzero(used == 0)[0]
    if ovf.size:
        xo = x[ovf]
        fb = _gelu_exact(xo @ Wf1 + bf1) @ Wf2 + bf2
        out[ovf] = fb.astype(np.float32)

    return out.astype(np.float32)


# revision 17
# speedup vs baseline: 5660.9137x; 1.0276x over previous
"""MoE FFN (capacity-gated routing) on 8 Trainium2 NeuronCores.

Strategy
--------
Expert-parallel: 16 experts / 8 cores = 2 experts per core. With full
host-side I/O, the dispatch (which token goes to which expert slot) IS
the sharding step, so routing runs on the host in numpy; each core
receives only the tokens routed to its two experts, pre-gathered,
pre-transposed to d-major layout, and pre-packed into the exact
per-partition SBUF byte order so every weight DMA is a contiguous
[128 x 4KB] transfer at full HBM bandwidth. The device runs the grouped
FFN (x @ W1 -> exact GELU -> @ W2, biases fused into ScalarE
activations as per-partition vectors) in bf16 with f32 PSUM
accumulation. Weight DMAs are chunked (4 chunks per matrix) so the
TensorE starts ~2us into the kernel and weights stream concurrently
with compute. The combine (scatter-add + 1/n_experts scaling) is a
cheap host-side gather. Tokens that overflow every attempted expert's
capacity (used == 0; zero of them for uniform-random routes) take the
fallback FFN, computed exactly on the host.
"""

import sys

if "/opt/trn_rl_repo" not in sys.path:
    sys.path.append("/opt/trn_rl_repo")

import numpy as np
import ml_dtypes

import concourse.tile as tile
from concourse import bacc, mybir
from concourse.bass_utils import run_bass_kernel_spmd

# Problem shape (hardcoded per contract)
D = 512        # d_model
F = 2048       # d_ff
E = 16         # num experts
B = 2048       # max tokens
CAP = 320      # per-expert capacity = int(1.25 * ceil(B * 2 / E))
N_CORES = 8
EPC = E // N_CORES  # experts per core

P = 128
KT = D // P    # k-tiles over d_model
FT = F // P    # tiles over d_ff
CH1 = 8        # w1 chunks along f
CH2 = 4        # w2 chunks along t (contraction)
M_PER = FT // CH1
T_PER = FT // CH2

BF16 = mybir.dt.bfloat16
F32 = mybir.dt.float32
NPBF16 = ml_dtypes.bfloat16

_CACHE = {}


def _build_nc(rep=1, act=None, nb=CAP, with_bias=True):
    """Per-core program: grouped FFN for EPC experts over CAP token slots.

    All tensors are d-major (partition dim = feature dim); weights arrive
    pre-packed in per-partition byte order (see _prep_in_maps), so every
    DMA is [128 x contiguous]. rep>1 repeats the body (timing); act
    overrides the GELU (CoreSim lacks Gelu; Identity is cost-equivalent).
    """
    if act is None:
        act = mybir.ActivationFunctionType.Gelu
    nc = bacc.Bacc(None)
    xg = nc.declare_dram_parameter("xg", [EPC, P, KT * nb], BF16, isOutput=False)
    w1 = nc.declare_dram_parameter(
        "w1", [EPC, CH1, P, KT * M_PER * P], BF16, isOutput=False
    )
    w2 = nc.declare_dram_parameter(
        "w2", [EPC, CH2, P, T_PER * D], BF16, isOutput=False
    )
    bb = (
        nc.declare_dram_parameter("bb", [EPC, F + D], F32, isOutput=False)
        if with_bias
        else None
    )
    yt = nc.declare_dram_parameter("yt", [EPC, P, KT * nb], F32, isOutput=True)

    with (
        tile.TileContext(nc) as tc,
        tc.tile_pool(name="consts", bufs=1) as consts,
        tc.tile_pool(name="xgp", bufs=2) as xgp,
        tc.tile_pool(name="w1p", bufs=2 * CH1 + 1) as w1p,
        tc.tile_pool(name="w2p", bufs=2 * CH2 + 1) as w2p,
        tc.tile_pool(name="hp", bufs=2) as hp,
        tc.tile_pool(name="yp", bufs=2) as yp,
        tc.tile_pool(name="ps1", bufs=4, space="PSUM") as ps1,
        tc.tile_pool(name="ps2", bufs=4, space="PSUM") as ps2,
    ):
        bb_sb = consts.tile([P, EPC, FT + KT], F32, name="bb_sb") if with_bias else None
        bias_issued = not with_bias

        for _ in range(rep):
            for e in range(EPC):
                PRE = 1  # w1 chunks issued ahead of xg
                w1cs = []
                for c in range(CH1):
                    w1c = w1p.tile([P, KT, M_PER * P], BF16, tag="w1c", name="w1c")
                    if c < PRE:
                        nc.sync.dma_start(
                            out=w1c,
                            in_=w1[e, c].rearrange("p (t f) -> p t f", t=KT),
                        )
                    w1cs.append(w1c)
                xg_sb = xgp.tile([P, KT, nb], BF16, name="xg_sb")
                nc.sync.dma_start(
                    out=xg_sb, in_=xg[e].rearrange("p (t n) -> p t n", t=KT)
                )
                h_sb = hp.tile([P, FT, nb], BF16, name="h_sb")

                # phase 1: h = gelu(x @ W1 + b1); w1 chunked along f so the
                # first matmuls start after one 512 KB chunk lands
                for c in range(CH1):
                    w1c = w1cs[c]
                    if c >= PRE:
                        nc.sync.dma_start(
                            out=w1c,
                            in_=w1[e, c].rearrange("p (t f) -> p t f", t=KT),
                        )
                    for ml in range(M_PER):
                        m = c * M_PER + ml
                        ps = ps1.tile([P, nb], F32, name="ps")
                        for k in range(KT):
                            nc.tensor.matmul(
                                ps,
                                w1c[:, k, ml * P : (ml + 1) * P],
                                xg_sb[:, k, :],
                                start=(k == 0),
                                stop=(k == KT - 1),
                            )
                        if not bias_issued:
                            # traced before the first ACT (deps follow trace
                            # order) but after the critical xg/w1c0 loads
                            nc.sync.dma_start(
                                out=bb_sb, in_=bb.rearrange("e (t p) -> p e t", p=P)
                            )
                            bias_issued = True
                        nc.scalar.activation(
                            h_sb[:, m, :],
                            ps,
                            act,
                            bias=bb_sb[:, e, m : m + 1] if with_bias else 0.0,
                        )

                # phase 2: y = h @ W2 + b2; t-outer so each w2 chunk is
                # consumed on arrival; the 4 output groups accumulate in
                # parallel across the 4 ps2 banks
                ps_y = [
                    ps2.tile([P, nb], F32, tag="psy", name=f"psy{m2}")
                    for m2 in range(KT)
                ]
                w2cs = []
                for c in range(CH2):
                    w2c = w2p.tile([P, T_PER, D], BF16, tag="w2c", name="w2c")
                    nc.sync.dma_start(
                        out=w2c,
                        in_=w2[e, c].rearrange("p (t d) -> p t d", t=T_PER),
                    )
                    w2cs.append(w2c)

                def mm2(t, m2):
                    c, tl = divmod(t, T_PER)
                    nc.tensor.matmul(
                        ps_y[m2],
                        w2cs[c][:, tl, m2 * P : (m2 + 1) * P],
                        h_sb[:, t, :],
                        start=(t == 0),
                        stop=(t == FT - 1),
                    )

                y_sb = yp.tile([P, KT, nb], F32, name="y_sb")

                def act_y(m2):
                    nc.scalar.activation(
                        y_sb[:, m2, :],
                        ps_y[m2],
                        mybir.ActivationFunctionType.Identity,
                        bias=(
                            bb_sb[:, e, FT + m2 : FT + m2 + 1] if with_bias else 0.0
                        ),
                    )

                last = e == EPC - 1
                half = FT // 2
                for t in range(half if last else FT):
                    for m2 in range(KT):
                        mm2(t, m2)
                if not last:
                    for m2 in range(KT):
                        act_y(m2)
                    nc.sync.dma_start(
                        out=yt[e].rearrange("p (t n) -> p t n", t=KT), in_=y_sb
                    )
                else:
                    # m2-outer second half: groups retire one by one so the
                    # two half-DMAs overlap the tail matmuls
                    for m2 in range(KT):
                        for t in range(half, FT):
                            mm2(t, m2)
                        act_y(m2)
                        if m2 == 1:
                            nc.sync.dma_start(
                                out=yt[e].rearrange("p (t n) -> p t n", t=KT)[
                                    :, 0:2, :
                                ],
                                in_=y_sb[:, 0:2, :],
                            )
                    nc.sync.dma_start(
                        out=yt[e].rearrange("p (t n) -> p t n", t=KT)[:, 2:4, :],
                        in_=y_sb[:, 2:4, :],
                    )
    nc.finalize()
    return nc


def get_nc(rep=1, act=None, nb=CAP, with_bias=True):
    key = (rep, act, nb, with_bias)
    if key not in _CACHE:
        _CACHE[key] = _build_nc(rep, act, nb, with_bias)
    return _CACHE[key]


def _bucket(routing):
    """Token-block width: smallest multiple of 32 covering the max expert
    load (valid slots are dense per expert), floored at 192 to bound the
    number of distinct compiled programs."""
    selected = routing[3]
    maxload = int(selected.sum(axis=0).max())
    nb = max(192, -(-maxload // 32) * 32)
    return min(CAP, nb)


def _route_np(routes):
    """Numpy replica of the reference's capacity-gated routing."""
    e_map = (routes.astype(np.int64) * E) // B                  # [B, K]
    sel0 = np.zeros((B, E), bool)
    np.put_along_axis(sel0, e_map, True, axis=1)
    sel0_i = sel0.astype(np.int32)
    cum = np.cumsum(sel0_i, axis=0) - sel0_i                    # exclusive cumsum
    selected = sel0 & (cum < CAP)
    slot = cum
    used = selected.sum(axis=1)
    tok_of_slot = np.zeros(E * CAP, np.int32)
    valid = np.zeros(E * CAP, bool)
    b_idx, e_idx = np.nonzero(selected)
    flat = e_idx * CAP + slot[b_idx, e_idx]
    tok_of_slot[flat] = b_idx
    valid[flat] = True
    return tok_of_slot, valid, used, selected, slot


def _pack_w1(W1e):
    # [D, F] -> [CH1, P, KT*M_PER*P]; [c, p, k*512 + x] = W1e[k*128+p, c*512+x]
    return np.ascontiguousarray(
        W1e.reshape(KT, P, CH1, M_PER * P).transpose(2, 1, 0, 3).reshape(CH1, P, -1)
    )


def _pack_w2(W2e):
    # [F, D] -> [CH2, P, T_PER*D]; [c, p, tl*512 + d] = W2e[(c*T_PER+tl)*128+p, d]
    return np.ascontiguousarray(
        W2e.reshape(CH2, T_PER, P, D).transpose(0, 2, 1, 3).reshape(CH2, P, -1)
    )


def _prep_in_maps(x, W1, b1, W2, b2, routing, nb=CAP, with_bias=True):
    tok_of_slot, valid, used, selected, slot = routing
    xg = x[tok_of_slot] * valid[:, None].astype(np.float32)     # [E*CAP, D]
    xgT = xg.reshape(E, CAP, D).transpose(0, 2, 1)[:, :, :nb]   # [E, D, nb]
    xg_pk = np.ascontiguousarray(
        xgT.reshape(E, KT, P, nb).transpose(0, 2, 1, 3).reshape(E, P, -1)
    ).astype(NPBF16)
    w1_pk = W1.astype(NPBF16)
    w2_pk = W2.astype(NPBF16)
    in_maps = []
    for c in range(N_CORES):
        sl = slice(c * EPC, (c + 1) * EPC)
        in_maps.append(
            {
                "xg": xg_pk[sl],
                "w1": np.stack([_pack_w1(w1_pk[i]) for i in range(sl.start, sl.stop)]),
                "w2": np.stack([_pack_w2(w2_pk[i]) for i in range(sl.start, sl.stop)]),
            }
        )
        if with_bias:
            in_maps[-1]["bb"] = np.ascontiguousarray(
                np.concatenate([b1[sl], b2[sl]], axis=1)
            )
    return in_maps


def _erf(v):
    # Abramowitz & Stegun 7.1.26, |err| <= 1.5e-7
    s = np.sign(v)
    a = np.abs(v)
    t = 1.0 / (1.0 + 0.3275911 * a)
    poly = t * (
        0.254829592
        + t * (-0.284496736 + t * (1.421413741 + t * (-1.453152027 + t * 1.061405429)))
    )
    return s * (1.0 - poly * np.exp(-a * a))


def _gelu_exact(v):
    return 0.5 * v * (1.0 + _erf(v / np.sqrt(2.0)))


def kernel(x, W1, b1, W2, b2, Wf1, bf1, Wf2, bf2, routes):
    x = np.asarray(x, np.float32)
    W1 = np.asarray(W1, np.float32)
    b1 = np.asarray(b1, np.float32)
    W2 = np.asarray(W2, np.float32)
    b2 = np.asarray(b2, np.float32)
    Wf1 = np.asarray(Wf1, np.float32)
    bf1 = np.asarray(bf1, np.float32)
    Wf2 = np.asarray(Wf2, np.float32)
    bf2 = np.asarray(bf2, np.float32)
    routes = np.asarray(routes)

    routing = _route_np(routes)
    tok_of_slot, valid, used, selected, slot = routing
    nb = _bucket(routing)
    with_bias = bool(np.any(b1) or np.any(b2))
    in_maps = _prep_in_maps(x, W1, b1, W2, b2, routing, nb, with_bias)

    nc = get_nc(nb=nb, with_bias=with_bias)
    res = run_bass_kernel_spmd(nc, in_maps, core_ids=list(range(N_CORES)))

    # [E, D, CAP] expert outputs (garbage in invalid slots; never read there)
    Y = np.concatenate([r["yt"] for r in res.results], axis=0)
    Y = Y.reshape(E, P, KT, nb).transpose(0, 2, 1, 3).reshape(E, D, nb)

    # Combine: each token was selected by <= 2 experts; gather its slot
    # outputs and average. Pure host-side gather.
    b_idx, e_idx = np.nonzero(selected)                         # ordered by token
    first = np.concatenate(([True], b_idx[1:] != b_idx[:-1]))
    s_of = slot[b_idx, e_idx]
    e1 = np.zeros(B, np.int64); c1 = np.zeros(B, np.int64); g1 = np.zeros(B, np.float32)
    e2 = np.zeros(B, np.int64); c2 = np.zeros(B, np.int64); g2 = np.zeros(B, np.float32)
    e1[b_idx[first]] = e_idx[first]; c1[b_idx[first]] = s_of[first]; g1[b_idx[first]] = 1.0
    e2[b_idx[~first]] = e_idx[~first]; c2[b_idx[~first]] = s_of[~first]; g2[b_idx[~first]] = 1.0
    out_sum = g1[:, None] * Y[e1, :, c1] + g2[:, None] * Y[e2, :, c2]
    inv = (1.0 / np.maximum(used, 1)).astype(np.float32)
    out = out_sum * inv[:, None]

    # Overflow tokens (used == 0): exact fallback FFN on host.
    ovf = np.nonzero(used == 0)[0]
    if ovf.size:
        xo = x[ovf]
        fb = _gelu_exact(xo @ Wf1 + bf1) @ Wf2 + bf2
        out[ovf] = fb.astype(np.float32)

    return out.astype(np.float32)
